# revision 1
# baseline (speedup 1.0000x reference)
"""Trainium2 Bass kernel for a GCN critic network (2x GCNConv + host readout).

Strategy: graphs are 32 nodes with no cross-graph edges, so the GCN
scatter-add is a dense 32x32 normalized-adjacency matmul per graph. Four
graphs pack into one 128x128 block-diagonal stationary operand for the
TensorEngine. Data-parallel over 8 cores (512 graphs / 128 blocks per core).

Per-core device pipeline (bf16 matmuls, fp32 PSUM accumulation):
  P1: A0t[f,d] per block  (lhsT=x_b [s,f], rhs=Ablk_b [s,d])
  P2: h1t = relu(W1^T @ A0t)   (feat-major, W1 halves stationary)
  P3: g_b = h1_b @ W2          (node-major, h1t halves as stationary)
  P4: agg2_b = Ablk_b^T @ g_b  (node-major), h2 = relu(agg2)
  RO: v[p,b] = sum_c h2*Mask;  out[j,b] = sum_{p in grp j} v[p,b] (G4 matmul)

All device inputs are packed into one DRAM tensor ("mega") loaded by four
quarter DMAs: each quarter holds the A-blocks then x-blocks for 32 blocks,
and the constant tail (W1, W2 halves, mask, G4) rides with quarter 3. This
keeps every consumer at <=1 DMA-sem wait (the compiler caps sync waits per
instruction at 2).
"""

from contextlib import ExitStack

import numpy as np
import ml_dtypes

NG = 4096
NPG = 32
NH = 13
IN_DIM = 128
H1 = 256
H2 = 64
NCORES = 8
GPC = NG // NCORES          # graphs per core = 512
BPC = GPC // 4              # blocks per core = 128 (4 graphs / block)
NPC = GPC * NPG             # nodes per core = 16384

# mega layout (elements per partition, bf16)
QBLK = BPC // 4             # 32 blocks per quarter
QA = QBLK * 128             # 4096: A-slab elems per quarter
QSTRIDE = 2 * QA            # 8192: quarter stride (A slab + x slab)
OFF_W1 = 4 * QSTRIDE        # 32768
OFF_W2 = OFF_W1 + H1        # 33024 (two [128,64] halves side by side)
OFF_MASK = OFF_W2 + 2 * H2  # 33152
OFF_G4 = OFF_MASK + H2      # 33216
MEGA_W = OFF_G4 + 4         # 33220

BF16 = ml_dtypes.bfloat16

_CACHE = {}


def _a_off(b):
    return (b // QBLK) * QSTRIDE + (b % QBLK) * 128


def _x_off(b):
    return (b // QBLK) * QSTRIDE + QA + (b % QBLK) * 128


def _build_bass():
    import concourse.bass as bass
    import concourse.mybir as mybir
    import concourse.tile as tile
    from concourse import bacc
    from concourse.bass import ds

    bf = mybir.dt.bfloat16
    f32 = mybir.dt.float32
    AF = mybir.ActivationFunctionType
    ALU = mybir.AluOpType

    nc = bacc.Bacc("TRN2", target_bir_lowering=False, debug=False)

    megadev = nc.declare_dram_parameter("megadev", [128, MEGA_W], bf, isOutput=False)
    outdev = nc.declare_dram_parameter("outdev", [4, BPC], f32, isOutput=True)

    with tile.TileContext(nc) as tc:
        ctx = ExitStack()
        ppool = ctx.enter_context(tc.tile_pool(name="persist", bufs=1))
        mega = ppool.tile([128, MEGA_W], bf, name="mega", tag="mega")
        a0t = ppool.tile([128, BPC * 128], bf, name="a0t", tag="a0t")
        h1a = ppool.tile([128, BPC * 128], bf, name="h1a", tag="h1a")
        h1b = ppool.tile([128, BPC * 128], bf, name="h1b", tag="h1b")
        v_all = ppool.tile([128, BPC], f32, name="v_all", tag="v_all")
        vb = ppool.tile([128, BPC], bf, name="vb", tag="vb")
        mask8 = ppool.tile([128, 8 * H2], bf, name="mask8", tag="mask8")
        out_sb = ppool.tile([4, BPC], f32, name="out_sb", tag="out_sb")

        w1at = mega[:, ds(OFF_W1, 128)]
        w1bt = mega[:, ds(OFF_W1 + 128, 128)]
        w2at = mega[:, ds(OFF_W2, H2)]
        w2bt = mega[:, ds(OFF_W2 + H2, H2)]
        maskc = mega[:, ds(OFF_MASK, H2)]
        g4t = mega[:, ds(OFF_G4, 4)]

        # quarter DMAs; tail consts ride with quarter 3
        for q in range(4):
            n = QSTRIDE if q < 3 else (MEGA_W - 3 * QSTRIDE)
            nc.sync.dma_start(mega[:, ds(q * QSTRIDE, n)], megadev[:, ds(q * QSTRIDE, n)])

        with tc.tile_pool(name="ps", bufs=1, space="PSUM") as pspool, \
             tc.tile_pool(name="ev", bufs=3) as evpool:

            # replicate mask 8x (DVE, waits only quarter-3 DMA)
            for j in range(8):
                nc.vector.tensor_scalar_add(mask8[:, ds(j * H2, H2)], maskc, 0.0)

            # ---- P1: A0t per block ----
            for i in range(BPC // 4):
                ps = pspool.tile([128, 512], f32, tag="ps", bufs=3)
                for j in range(4):
                    b = 4 * i + j
                    nc.tensor.matmul(
                        ps[:, ds(128 * j, 128)],
                        lhsT=mega[:, ds(_x_off(b), 128)],
                        rhs=mega[:, ds(_a_off(b), 128)],
                        start=True, stop=True,
                    )
                dst = a0t[:, ds(i * 512, 512)]
                if i % 2 == 0:
                    nc.vector.tensor_scalar_add(dst, ps[:], 0.0)
                else:
                    nc.scalar.copy(dst, ps[:])

            # ---- P2: h1t halves, relu fused in eviction ----
            for i in range(BPC // 4):
                src = a0t[:, ds(i * 512, 512)]
                ps1 = pspool.tile([128, 512], f32, tag="ps", bufs=3)
                nc.tensor.matmul(ps1[:], lhsT=w1at, rhs=src, start=True, stop=True)
                dst1 = h1a[:, ds(i * 512, 512)]
                ps2 = pspool.tile([128, 512], f32, tag="ps", bufs=3)
                nc.tensor.matmul(ps2[:], lhsT=w1bt, rhs=src, start=True, stop=True)
                dst2 = h1b[:, ds(i * 512, 512)]
                if i % 2 == 0:
                    nc.scalar.activation(dst1, ps1[:], AF.Relu)
                    nc.vector.tensor_scalar_max(dst2, ps2[:], 0.0)
                else:
                    nc.vector.tensor_scalar_max(dst1, ps1[:], 0.0)
                    nc.scalar.activation(dst2, ps2[:], AF.Relu)

            # ---- P3+P4+readout per 8-block group ----
            for i in range(BPC // 8):
                gps = pspool.tile([128, 512], f32, tag="ps", bufs=3)
                for j in range(8):
                    b = 8 * i + j
                    nsl = ds(b * 128, 128)
                    osl = ds(j * H2, H2)
                    nc.tensor.matmul(gps[:, osl], lhsT=h1a[:, nsl], rhs=w2at,
                                     start=True, stop=False)
                    nc.tensor.matmul(gps[:, osl], lhsT=h1b[:, nsl], rhs=w2bt,
                                     start=False, stop=True)
                g8 = evpool.tile([128, 512], bf, tag="g8")
                nc.scalar.copy(g8[:], gps[:])

                aps = pspool.tile([128, 512], f32, tag="ps", bufs=3)
                for j in range(8):
                    b = 8 * i + j
                    nc.tensor.matmul(aps[:, ds(j * H2, H2)],
                                     lhsT=mega[:, ds(_a_off(b), 128)],
                                     rhs=g8[:, ds(j * H2, H2)],
                                     start=True, stop=True)
                h28 = evpool.tile([128, 512], bf, tag="h28")
                nc.vector.tensor_scalar_max(h28[:], aps[:], 0.0)

                mm8 = evpool.tile([128, 512], bf, tag="mm8")
                nc.vector.scalar_tensor_tensor(
                    mm8[:], h28[:], 0.0, mask8[:],
                    op0=ALU.add, op1=ALU.mult,
                )
                nc.vector.tensor_reduce(
                    v_all[:, ds(i * 8, 8)],
                    mm8[:].rearrange("p (j c) -> p j c", c=H2),
                    axis=mybir.AxisListType.X,
                    op=ALU.add,
                )

            # ---- final per-graph group reduce ----
            nc.vector.tensor_scalar_add(vb[:], v_all[:], 0.0)
            ops = pspool.tile([4, BPC], f32, tag="ops", bufs=1)
            nc.tensor.matmul(ops[:], lhsT=g4t, rhs=vb[:], start=True, stop=True)
            nc.scalar.copy(out_sb[:], ops[:])
            nc.sync.dma_start(outdev[:, :], out_sb[:])

        ctx.close()

    nc.compile()
    return nc


def _prep_inputs(x, ei, host_idx, W1, b1, W2, b2, Wout, bout):
    """Host-side: dense per-graph adjacency, packed layouts, sharding.
    Returns (in_maps, bout_val) or None if structural assumptions fail."""
    x = np.asarray(x); ei = np.asarray(ei); host_idx = np.asarray(host_idx)
    W1 = np.asarray(W1); b1 = np.asarray(b1); W2 = np.asarray(W2)
    b2 = np.asarray(b2); Wout = np.asarray(Wout); bout = np.asarray(bout)

    N = NG * NPG
    src = ei[0].astype(np.int64)
    dst = ei[1].astype(np.int64)
    if (src // NPG != dst // NPG).any():
        return None
    hi = host_idx.reshape(NG, NH)
    if not (hi == (np.arange(NG)[:, None] * NPG + np.arange(NH)[None, :])).all():
        return None
    if b1.any() or b2.any():
        return None

    deg = np.bincount(dst, minlength=N).astype(np.float64) + 1.0
    dinv = 1.0 / np.sqrt(deg)
    A = np.zeros((NG, NPG, NPG), dtype=np.float64)
    g = src // NPG
    np.add.at(A, (g, dst % NPG, src % NPG), dinv[src] * dinv[dst])
    A[:, np.arange(NPG), np.arange(NPG)] += (dinv * dinv).reshape(NG, NPG)
    A32 = A.astype(np.float32)

    WoutR = Wout[:, 0].reshape(NH, H2).astype(np.float32)
    mask = np.zeros((128, H2), dtype=np.float32)
    for p in range(128):
        if p % NPG < NH:
            mask[p] = WoutR[p % NPG]
    g4 = np.zeros((128, 4), dtype=np.float32)
    for p in range(128):
        g4[p, p // 32] = 1.0

    w2re = np.empty((128, 2 * H2), dtype=np.float32)
    w2re[:, :H2] = W2[:128]
    w2re[:, H2:] = W2[128:]

    in_maps = []
    for c in range(NCORES):
        mega = np.zeros((128, MEGA_W), dtype=np.float32)
        xc = x[c * NPC:(c + 1) * NPC].reshape(BPC, 128, IN_DIM)
        xc = np.ascontiguousarray(xc.transpose(1, 0, 2))       # [128, BPC, 128]
        Ac = A32[c * GPC:(c + 1) * GPC].reshape(BPC, 4, NPG, NPG)
        Ablk = np.zeros((BPC, 128, 128), dtype=np.float32)
        for j in range(4):
            # Ablk[b][s, d] = A[g][d_local, s_local]  (transposed within graph)
            Ablk[:, 32 * j:32 * (j + 1), 32 * j:32 * (j + 1)] = \
                Ac[:, j].transpose(0, 2, 1)
        Ablk = np.ascontiguousarray(Ablk.transpose(1, 0, 2))   # [128, BPC, 128]
        for q in range(4):
            bs = slice(q * QBLK, (q + 1) * QBLK)
            mega[:, q * QSTRIDE:q * QSTRIDE + QA] = \
                Ablk[:, bs].reshape(128, QA)
            mega[:, q * QSTRIDE + QA:(q + 1) * QSTRIDE] = \
                xc[:, bs].reshape(128, QA)
        mega[:, OFF_W1:OFF_W1 + H1] = W1
        mega[:, OFF_W2:OFF_W2 + 2 * H2] = w2re
        mega[:, OFF_MASK:OFF_MASK + H2] = mask
        mega[:, OFF_G4:OFF_G4 + 4] = g4
        in_maps.append({"megadev": mega.astype(BF16)})
    return in_maps, float(bout[0])


def _numpy_fallback(x, ei, host_idx, W1, b1, W2, b2, Wout, bout):
    import jax
    jax.config.update("jax_platforms", "cpu")
    import jax.numpy as jnp

    def gcn_conv(xx, eei, W, b):
        Nn = xx.shape[0]
        loop = jnp.arange(Nn, dtype=eei.dtype)
        s = jnp.concatenate([eei[0], loop])
        d = jnp.concatenate([eei[1], loop])
        deg = jax.ops.segment_sum(jnp.ones(d.shape, dtype=xx.dtype), d, num_segments=Nn)
        dinv = jnp.where(deg > 0, jax.lax.rsqrt(deg), 0.0)
        norm = dinv[s] * dinv[d]
        h = xx @ W
        agg = jax.ops.segment_sum(h[s] * norm[:, None], d, num_segments=Nn)
        return agg + b

    h = jax.nn.relu(gcn_conv(jnp.asarray(x), jnp.asarray(ei), jnp.asarray(W1), jnp.asarray(b1)))
    h = jax.nn.relu(gcn_conv(h, jnp.asarray(ei), jnp.asarray(W2), jnp.asarray(b2)))
    host_z = h[jnp.asarray(host_idx)]
    nb = host_idx.shape[0] // NH
    z = host_z.reshape(nb, NH * h.shape[1])
    return np.asarray(z @ jnp.asarray(Wout) + jnp.asarray(bout))


def kernel(**inputs):
    prep = _prep_inputs(**inputs)
    if prep is None:
        return _numpy_fallback(**inputs)
    in_maps, bout_val = prep

    from concourse.bass_utils import run_bass_kernel_spmd

    if "nc" not in _CACHE:
        _CACHE["nc"] = _build_bass()
    nc = _CACHE["nc"]

    res = run_bass_kernel_spmd(nc, in_maps, core_ids=list(range(NCORES)))
    out = np.empty((NG, 1), dtype=np.float32)
    for c in range(NCORES):
        o = res.results[c]["outdev"]          # [4, BPC]
        out[c * GPC:(c + 1) * GPC, 0] = o.T.ravel()
    out += bout_val
    return out



# revision 2
# speedup vs baseline: 1.2491x; 1.2491x over previous
"""Trainium2 Bass kernel for a GCN critic network (2x GCNConv + host readout).

Strategy: graphs are 32 nodes with no cross-graph edges, so the GCN
scatter-add is a dense 32x32 normalized-adjacency matmul per graph. Four
graphs pack into one 128x128 block-diagonal stationary operand for the
TensorEngine. Data-parallel over 8 cores (512 graphs / 128 blocks per core).

v2: chunked wavefront. The per-core input is one DRAM tensor ("mega") with
the small constants FIRST, then 16 chunks of 8 blocks ([A slab | x slab]).
Each chunk is loaded by its own DMA so compute starts as soon as chunk 0
lands and stays pipelined behind the DMA wave. Per chunk the full chain
runs back to back (bf16 matmuls, fp32 PSUM):
  P1: a0t[f,d] per block      (lhsT=x_b [s,f], rhs=Ablk_b [s,d])
  P2: h1t = relu(W1^T @ a0t)  (feat-major, W1 halves stationary)
  P3: g_b = h1_b @ W2         (node-major, h1t halves stationary)
  P4: h2 = Ablk_b^T @ g_b     (node-major)
  RO: mm = max(psum,0)*mask (relu fused into the PSUM read), then a
      free-axis reduce per 64-col group -> v[p, block].
PSUM evictions are the co-bottleneck with the PE (GPSIMD has no PSUM
port), so they are split across Scalar (h1 relu, half of g) and Vector
(a0t, other half of g, STT, reduce) to balance the two engines.
"""

from contextlib import ExitStack

import numpy as np
import ml_dtypes

NG = 4096
NPG = 32
NH = 13
IN_DIM = 128
H1 = 256
H2 = 64
NCORES = 8
GPC = NG // NCORES          # graphs per core = 512
BPC = GPC // 4              # blocks per core = 128 (4 graphs / block)
NPC = GPC * NPG             # nodes per core = 16384

# mega layout (elements per partition, bf16): consts first, then chunks
NCHUNK = 16
CBLK = BPC // NCHUNK        # 8 blocks per chunk
CA = CBLK * 128             # 1024: A-slab elems per chunk
CSTRIDE = 2 * CA            # 2048: chunk stride (A slab + x slab)
OFF_W1 = 0                  # [128, 256]
OFF_W2 = OFF_W1 + H1        # two [128,64] halves side by side
OFF_MASK = OFF_W2 + 2 * H2  # [128, 64]
OFF_G4 = OFF_MASK + H2      # [128, 4]
CONST_W = 512
MEGA_W = CONST_W + NCHUNK * CSTRIDE  # 33280

BF16 = ml_dtypes.bfloat16

_CACHE = {}


def _a_off(b):
    return CONST_W + (b // CBLK) * CSTRIDE + (b % CBLK) * 128


def _x_off(b):
    return CONST_W + (b // CBLK) * CSTRIDE + CA + (b % CBLK) * 128


def _build_bass():
    import concourse.bass as bass
    import concourse.mybir as mybir
    import concourse.tile as tile
    from concourse import bacc
    from concourse.bass import ds

    bf = mybir.dt.bfloat16
    f32 = mybir.dt.float32
    AF = mybir.ActivationFunctionType
    ALU = mybir.AluOpType

    nc = bacc.Bacc("TRN2", target_bir_lowering=False, debug=False)

    megadev = nc.declare_dram_parameter("megadev", [128, MEGA_W], bf, isOutput=False)
    outdev = nc.declare_dram_parameter("outdev", [4, BPC], f32, isOutput=True)

    with tile.TileContext(nc) as tc:
        ctx = ExitStack()
        ppool = ctx.enter_context(tc.tile_pool(name="persist", bufs=1))
        mega = ppool.tile([128, MEGA_W], bf, name="mega", tag="mega")
        v_all = ppool.tile([128, BPC], f32, name="v_all", tag="v_all")
        vb = ppool.tile([128, BPC], bf, name="vb", tag="vb")
        mask8 = ppool.tile([128, 8 * H2], bf, name="mask8", tag="mask8")
        out_sb = ppool.tile([4, BPC], f32, name="out_sb", tag="out_sb")

        w1at = mega[:, ds(OFF_W1, 128)]
        w1bt = mega[:, ds(OFF_W1 + 128, 128)]
        w2at = mega[:, ds(OFF_W2, H2)]
        w2bt = mega[:, ds(OFF_W2 + H2, H2)]
        maskc = mega[:, ds(OFF_MASK, H2)]
        g4t = mega[:, ds(OFF_G4, 4)]

        # consts first, then one DMA per chunk, issued in consumption order
        nc.sync.dma_start(mega[:, ds(0, CONST_W)], megadev[:, ds(0, CONST_W)])
        for c in range(NCHUNK):
            off = CONST_W + c * CSTRIDE
            nc.sync.dma_start(mega[:, ds(off, CSTRIDE)], megadev[:, ds(off, CSTRIDE)])

        with tc.tile_pool(name="ps", bufs=1, space="PSUM") as pspool, \
             tc.tile_pool(name="ev", bufs=2) as evpool:

            # replicate mask 8x (DVE, waits only on the consts DMA)
            for j in range(8):
                nc.vector.tensor_scalar_add(mask8[:, ds(j * H2, H2)], maskc, 0.0)

            for c in range(NCHUNK):
                # ---- P1: a0t per block (8 matmuls) ----
                ps_a = pspool.tile([128, 1024], f32, tag="pA", bufs=1)
                for j in range(8):
                    b = CBLK * c + j
                    nc.tensor.matmul(
                        ps_a[:, ds(128 * j, 128)],
                        lhsT=mega[:, ds(_x_off(b), 128)],
                        rhs=mega[:, ds(_a_off(b), 128)],
                        start=True, stop=True,
                    )
                a0t = evpool.tile([128, 1024], bf, tag="a0t")
                nc.vector.tensor_scalar_add(a0t[:], ps_a[:], 0.0)

                # ---- P2: h1t halves, relu fused in eviction ----
                ps_h = pspool.tile([128, 2048], f32, tag="pH", bufs=1)
                nc.tensor.matmul(ps_h[:, ds(0, 512)], lhsT=w1at,
                                 rhs=a0t[:, ds(0, 512)], start=True, stop=True)
                nc.tensor.matmul(ps_h[:, ds(512, 512)], lhsT=w1bt,
                                 rhs=a0t[:, ds(0, 512)], start=True, stop=True)
                nc.tensor.matmul(ps_h[:, ds(1024, 512)], lhsT=w1at,
                                 rhs=a0t[:, ds(512, 512)], start=True, stop=True)
                nc.tensor.matmul(ps_h[:, ds(1536, 512)], lhsT=w1bt,
                                 rhs=a0t[:, ds(512, 512)], start=True, stop=True)
                h1 = evpool.tile([128, 2048], bf, tag="h1")
                nc.scalar.activation(h1[:], ps_h[:], AF.Relu)

                # ---- P3: g per block (16 matmuls, W2 halves moving) ----
                ps_g = pspool.tile([128, 512], f32, tag="pG", bufs=1)
                for j in range(8):
                    grp, o = j // 4, j % 4
                    la = h1[:, ds(grp * 1024 + o * 128, 128)]
                    lb = h1[:, ds(grp * 1024 + 512 + o * 128, 128)]
                    nc.tensor.matmul(ps_g[:, ds(j * H2, H2)], lhsT=la, rhs=w2at,
                                     start=True, stop=False)
                    nc.tensor.matmul(ps_g[:, ds(j * H2, H2)], lhsT=lb, rhs=w2bt,
                                     start=False, stop=True)
                g8 = evpool.tile([128, 512], bf, tag="g8")
                nc.scalar.copy(g8[:, ds(0, 256)], ps_g[:, ds(0, 256)])
                nc.vector.tensor_scalar_add(g8[:, ds(256, 256)], ps_g[:, ds(256, 256)], 0.0)

                # ---- P4: h2 = A^T g per block ----
                ps_r = pspool.tile([128, 512], f32, tag="pR", bufs=1)
                for j in range(8):
                    b = CBLK * c + j
                    nc.tensor.matmul(ps_r[:, ds(j * H2, H2)],
                                     lhsT=mega[:, ds(_a_off(b), 128)],
                                     rhs=g8[:, ds(j * H2, H2)],
                                     start=True, stop=True)

                # ---- readout: relu+mask fused from PSUM, then reduce ----
                mm = evpool.tile([128, 512], bf, tag="mm")
                nc.vector.scalar_tensor_tensor(
                    mm[:], ps_r[:], 0.0, mask8[:],
                    op0=ALU.max, op1=ALU.mult,
                )
                nc.vector.tensor_reduce(
                    v_all[:, ds(c * 8, 8)],
                    mm[:].rearrange("p (j c) -> p j c", c=H2),
                    axis=mybir.AxisListType.X,
                    op=ALU.add,
                )

            # ---- final per-graph group reduce ----
            nc.vector.tensor_scalar_add(vb[:], v_all[:], 0.0)
            ops = pspool.tile([128, 512], f32, tag="pG", bufs=1)
            nc.tensor.matmul(ops[0:4, ds(0, BPC)], lhsT=g4t, rhs=vb[:],
                             start=True, stop=True)
            nc.scalar.copy(out_sb[:], ops[0:4, ds(0, BPC)])
            nc.sync.dma_start(outdev[:, :], out_sb[:])

        ctx.close()

    nc.compile()
    return nc


def _prep_inputs(x, ei, host_idx, W1, b1, W2, b2, Wout, bout):
    """Host-side: dense per-graph adjacency, packed layouts, sharding.
    Returns (in_maps, bout_val) or None if structural assumptions fail."""
    x = np.asarray(x); ei = np.asarray(ei); host_idx = np.asarray(host_idx)
    W1 = np.asarray(W1); b1 = np.asarray(b1); W2 = np.asarray(W2)
    b2 = np.asarray(b2); Wout = np.asarray(Wout); bout = np.asarray(bout)

    N = NG * NPG
    src = ei[0].astype(np.int64)
    dst = ei[1].astype(np.int64)
    if (src // NPG != dst // NPG).any():
        return None
    hi = host_idx.reshape(NG, NH)
    if not (hi == (np.arange(NG)[:, None] * NPG + np.arange(NH)[None, :])).all():
        return None
    if b1.any() or b2.any():
        return None

    deg = np.bincount(dst, minlength=N).astype(np.float64) + 1.0
    dinv = 1.0 / np.sqrt(deg)
    A = np.zeros((NG, NPG, NPG), dtype=np.float64)
    g = src // NPG
    np.add.at(A, (g, dst % NPG, src % NPG), dinv[src] * dinv[dst])
    A[:, np.arange(NPG), np.arange(NPG)] += (dinv * dinv).reshape(NG, NPG)
    A32 = A.astype(np.float32)

    WoutR = Wout[:, 0].reshape(NH, H2).astype(np.float32)
    mask = np.zeros((128, H2), dtype=np.float32)
    for p in range(128):
        if p % NPG < NH:
            mask[p] = WoutR[p % NPG]
    g4 = np.zeros((128, 4), dtype=np.float32)
    for p in range(128):
        g4[p, p // 32] = 1.0

    w2re = np.empty((128, 2 * H2), dtype=np.float32)
    w2re[:, :H2] = W2[:128]
    w2re[:, H2:] = W2[128:]

    in_maps = []
    for c in range(NCORES):
        mega = np.zeros((128, MEGA_W), dtype=np.float32)
        mega[:, OFF_W1:OFF_W1 + H1] = W1
        mega[:, OFF_W2:OFF_W2 + 2 * H2] = w2re
        mega[:, OFF_MASK:OFF_MASK + H2] = mask
        mega[:, OFF_G4:OFF_G4 + 4] = g4
        xc = x[c * NPC:(c + 1) * NPC].reshape(BPC, 128, IN_DIM)
        xc = np.ascontiguousarray(xc.transpose(1, 0, 2))       # [128, BPC, 128]
        Ac = A32[c * GPC:(c + 1) * GPC].reshape(BPC, 4, NPG, NPG)
        Ablk = np.zeros((BPC, 128, 128), dtype=np.float32)
        for j in range(4):
            # Ablk[b][s, d] = A[g][d_local, s_local]  (transposed within graph)
            Ablk[:, 32 * j:32 * (j + 1), 32 * j:32 * (j + 1)] = \
                Ac[:, j].transpose(0, 2, 1)
        Ablk = np.ascontiguousarray(Ablk.transpose(1, 0, 2))   # [128, BPC, 128]
        for q in range(NCHUNK):
            bs = slice(q * CBLK, (q + 1) * CBLK)
            base = CONST_W + q * CSTRIDE
            mega[:, base:base + CA] = Ablk[:, bs].reshape(128, CA)
            mega[:, base + CA:base + CSTRIDE] = xc[:, bs].reshape(128, CA)
        in_maps.append({"megadev": mega.astype(BF16)})
    return in_maps, float(bout[0])


def _numpy_fallback(x, ei, host_idx, W1, b1, W2, b2, Wout, bout):
    import jax
    jax.config.update("jax_platforms", "cpu")
    import jax.numpy as jnp

    def gcn_conv(xx, eei, W, b):
        Nn = xx.shape[0]
        loop = jnp.arange(Nn, dtype=eei.dtype)
        s = jnp.concatenate([eei[0], loop])
        d = jnp.concatenate([eei[1], loop])
        deg = jax.ops.segment_sum(jnp.ones(d.shape, dtype=xx.dtype), d, num_segments=Nn)
        dinv = jnp.where(deg > 0, jax.lax.rsqrt(deg), 0.0)
        norm = dinv[s] * dinv[d]
        h = xx @ W
        agg = jax.ops.segment_sum(h[s] * norm[:, None], d, num_segments=Nn)
        return agg + b

    h = jax.nn.relu(gcn_conv(jnp.asarray(x), jnp.asarray(ei), jnp.asarray(W1), jnp.asarray(b1)))
    h = jax.nn.relu(gcn_conv(h, jnp.asarray(ei), jnp.asarray(W2), jnp.asarray(b2)))
    host_z = h[jnp.asarray(host_idx)]
    nb = host_idx.shape[0] // NH
    z = host_z.reshape(nb, NH * h.shape[1])
    return np.asarray(z @ jnp.asarray(Wout) + jnp.asarray(bout))


def kernel(**inputs):
    prep = _prep_inputs(**inputs)
    if prep is None:
        return _numpy_fallback(**inputs)
    in_maps, bout_val = prep

    from concourse.bass_utils import run_bass_kernel_spmd

    if "nc" not in _CACHE:
        _CACHE["nc"] = _build_bass()
    nc = _CACHE["nc"]

    res = run_bass_kernel_spmd(nc, in_maps, core_ids=list(range(NCORES)))
    out = np.empty((NG, 1), dtype=np.float32)
    for c in range(NCORES):
        o = res.results[c]["outdev"]          # [4, BPC]
        out[c * GPC:(c + 1) * GPC, 0] = o.T.ravel()
    out += bout_val
    return out


# revision 3
# speedup vs baseline: 1.2753x; 1.0209x over previous
"""Trainium2 Bass kernel for a GCN critic network (2x GCNConv + host readout).

Strategy: graphs are 32 nodes with no cross-graph edges, so the GCN
scatter-add is a dense 32x32 normalized-adjacency matmul per graph. Four
graphs pack into one 128x128 block-diagonal stationary operand for the
TensorEngine. Data-parallel over 8 cores (512 graphs / 128 blocks per core).

v2: chunked wavefront. The per-core input is one DRAM tensor ("mega") with
the small constants FIRST, then 16 chunks of 8 blocks ([A slab | x slab]).
Each chunk is loaded by its own DMA so compute starts as soon as chunk 0
lands and stays pipelined behind the DMA wave. Per chunk the full chain
runs back to back (bf16 matmuls, fp32 PSUM):
  P1: a0t[f,d] per block      (lhsT=x_b [s,f], rhs=Ablk_b [s,d])
  P2: h1t = relu(W1^T @ a0t)  (feat-major, W1 halves stationary)
  P3: g_b = h1_b @ W2         (node-major, h1t halves stationary)
  P4: h2 = Ablk_b^T @ g_b     (node-major)
  RO: mm = max(psum,0)*mask (relu fused into the PSUM read), then a
      free-axis reduce per 64-col group -> v[p, block].
PSUM evictions are the co-bottleneck with the PE (GPSIMD has no PSUM
port), so they are split across Scalar (h1 relu, half of g) and Vector
(a0t, other half of g, STT, reduce) to balance the two engines.
"""

from contextlib import ExitStack

import numpy as np
import ml_dtypes

NG = 4096
NPG = 32
NH = 13
IN_DIM = 128
H1 = 256
H2 = 64
NCORES = 8
GPC = NG // NCORES          # graphs per core = 512
BPC = GPC // 4              # blocks per core = 128 (4 graphs / block)
NPC = GPC * NPG             # nodes per core = 16384

# mega layout (elements per partition, bf16): consts first, then chunks
NCHUNK = 16
CBLK = BPC // NCHUNK        # 8 blocks per chunk
CA = CBLK * 128             # 1024: A-slab elems per chunk
CSTRIDE = 2 * CA            # 2048: chunk stride (A slab + x slab)
OFF_W1 = 0                  # [128, 256]
OFF_W2 = OFF_W1 + H1        # two [128,64] halves side by side
OFF_MASK = OFF_W2 + 2 * H2  # [128, 64]
OFF_G4 = OFF_MASK + H2      # [128, 4]
CONST_W = 512
MEGA_W = CONST_W + NCHUNK * CSTRIDE  # 33280

BF16 = ml_dtypes.bfloat16

_CACHE = {}


def _a_off(b):
    return CONST_W + (b // CBLK) * CSTRIDE + (b % CBLK) * 128


def _x_off(b):
    return CONST_W + (b // CBLK) * CSTRIDE + CA + (b % CBLK) * 128


def _build_bass():
    import concourse.bass as bass
    import concourse.mybir as mybir
    import concourse.tile as tile
    from concourse import bacc
    from concourse.bass import ds

    bf = mybir.dt.bfloat16
    f32 = mybir.dt.float32
    AF = mybir.ActivationFunctionType
    ALU = mybir.AluOpType

    nc = bacc.Bacc("TRN2", target_bir_lowering=False, debug=False)

    megadev = nc.declare_dram_parameter("megadev", [128, MEGA_W], bf, isOutput=False)
    outdev = nc.declare_dram_parameter("outdev", [4, BPC], f32, isOutput=True)

    with tile.TileContext(nc) as tc:
        ctx = ExitStack()
        ppool = ctx.enter_context(tc.tile_pool(name="persist", bufs=1))
        mega = ppool.tile([128, MEGA_W], bf, name="mega", tag="mega")
        v_all = ppool.tile([128, BPC], f32, name="v_all", tag="v_all")
        vb = ppool.tile([128, BPC], bf, name="vb", tag="vb")
        mask8 = ppool.tile([128, 8 * H2], bf, name="mask8", tag="mask8")
        out_sb = ppool.tile([4, BPC], f32, name="out_sb", tag="out_sb")

        w1at = mega[:, ds(OFF_W1, 128)]
        w1bt = mega[:, ds(OFF_W1 + 128, 128)]
        w2at = mega[:, ds(OFF_W2, H2)]
        w2bt = mega[:, ds(OFF_W2 + H2, H2)]
        maskc = mega[:, ds(OFF_MASK, H2)]
        g4t = mega[:, ds(OFF_G4, 4)]

        # consts first, then one DMA per chunk, issued in consumption order
        nc.sync.dma_start(mega[:, ds(0, CONST_W)], megadev[:, ds(0, CONST_W)])
        for c in range(NCHUNK):
            off = CONST_W + c * CSTRIDE
            nc.sync.dma_start(mega[:, ds(off, CSTRIDE)], megadev[:, ds(off, CSTRIDE)])

        with tc.tile_pool(name="ps", bufs=1, space="PSUM") as pspool, \
             tc.tile_pool(name="ev", bufs=2) as evpool:

            # replicate mask 8x (DVE, waits only on the consts DMA)
            for j in range(8):
                nc.vector.tensor_scalar_add(mask8[:, ds(j * H2, H2)], maskc, 0.0)

            # Software-pipelined over chunks: tick t emits P2 of chunk t-1,
            # P1 of chunk t, P3 of t-2, P4+readout of t-3 so each engine has
            # independent work every tick and cross-engine deps are a full
            # tick old. Evictions: ACT gets g[0:256] + h1 (in halves, chasing
            # P2's matmul pairs); DVE gets g[256:512], stt, reduce, then a0t
            # last (its producer P1 runs earlier in the same tick).
            a0t_t = {}
            h1_t = {}
            g8_t = {}
            for t in range(NCHUNK + 3):
                # ---- P2(t-1): h1t halves (4 matmuls) ----
                if 1 <= t <= NCHUNK:
                    c = t - 1
                    a0t = a0t_t.pop(c)
                    ps_h = pspool.tile([128, 2048], f32, tag="pH", bufs=1)
                    nc.tensor.matmul(ps_h[:, ds(0, 512)], lhsT=w1at,
                                     rhs=a0t[:, ds(0, 512)], start=True, stop=True)
                    nc.tensor.matmul(ps_h[:, ds(512, 512)], lhsT=w1bt,
                                     rhs=a0t[:, ds(0, 512)], start=True, stop=True)
                    h1 = evpool.tile([128, 2048], bf, tag="h1")
                    h1_t[c] = h1
                    nc.scalar.activation(h1[:, ds(0, 1024)], ps_h[:, ds(0, 1024)],
                                         AF.Relu)
                    nc.tensor.matmul(ps_h[:, ds(1024, 512)], lhsT=w1at,
                                     rhs=a0t[:, ds(512, 512)], start=True, stop=True)
                    nc.tensor.matmul(ps_h[:, ds(1536, 512)], lhsT=w1bt,
                                     rhs=a0t[:, ds(512, 512)], start=True, stop=True)
                    nc.scalar.activation(h1[:, ds(1024, 1024)], ps_h[:, ds(1024, 1024)],
                                         AF.Relu)

                # ---- P1(t): a0t per block (8 matmuls) ----
                if t < NCHUNK:
                    ps_a = pspool.tile([128, 1024], f32, tag="pA", bufs=1)
                    for j in range(8):
                        b = CBLK * t + j
                        nc.tensor.matmul(
                            ps_a[:, ds(128 * j, 128)],
                            lhsT=mega[:, ds(_x_off(b), 128)],
                            rhs=mega[:, ds(_a_off(b), 128)],
                            start=True, stop=True,
                        )

                # ---- P3(t-2): g per block (16 matmuls) ----
                if 2 <= t <= NCHUNK + 1:
                    c = t - 2
                    h1 = h1_t.pop(c)
                    ps_g = pspool.tile([128, 512], f32, tag="pG", bufs=1)
                    for j in range(8):
                        grp, o = j // 4, j % 4
                        la = h1[:, ds(grp * 1024 + o * 128, 128)]
                        lb = h1[:, ds(grp * 1024 + 512 + o * 128, 128)]
                        nc.tensor.matmul(ps_g[:, ds(j * H2, H2)], lhsT=la, rhs=w2at,
                                         start=True, stop=False)
                        nc.tensor.matmul(ps_g[:, ds(j * H2, H2)], lhsT=lb, rhs=w2bt,
                                         start=False, stop=True)
                    g8 = evpool.tile([128, 512], bf, tag="g8")
                    g8_t[c] = g8
                    nc.scalar.copy(g8[:, ds(0, 256)], ps_g[:, ds(0, 256)])
                    nc.vector.tensor_scalar_add(g8[:, ds(256, 256)],
                                                ps_g[:, ds(256, 256)], 0.0)

                # ---- P4(t-3) + readout ----
                if 3 <= t:
                    c = t - 3
                    g8 = g8_t.pop(c)
                    ps_r = pspool.tile([128, 512], f32, tag="pR", bufs=1)
                    for j in range(8):
                        b = CBLK * c + j
                        nc.tensor.matmul(ps_r[:, ds(j * H2, H2)],
                                         lhsT=mega[:, ds(_a_off(b), 128)],
                                         rhs=g8[:, ds(j * H2, H2)],
                                         start=True, stop=True)
                    mm = evpool.tile([128, 512], bf, tag="mm")
                    nc.vector.scalar_tensor_tensor(
                        mm[:], ps_r[:], 0.0, mask8[:],
                        op0=ALU.max, op1=ALU.mult,
                    )
                    nc.vector.tensor_reduce(
                        v_all[:, ds(c * 8, 8)],
                        mm[:].rearrange("p (j c) -> p j c", c=H2),
                        axis=mybir.AxisListType.X,
                        op=ALU.add,
                    )

                # ---- a0t eviction for P1(t), last on DVE ----
                if t < NCHUNK:
                    a0t = evpool.tile([128, 1024], bf, tag="a0t")
                    a0t_t[t] = a0t
                    nc.vector.tensor_scalar_add(a0t[:], ps_a[:], 0.0)

            # ---- final per-graph group reduce ----
            nc.vector.tensor_scalar_add(vb[:], v_all[:], 0.0)
            ops = pspool.tile([128, 512], f32, tag="pG", bufs=1)
            nc.tensor.matmul(ops[0:4, ds(0, BPC)], lhsT=g4t, rhs=vb[:],
                             start=True, stop=True)
            nc.scalar.copy(out_sb[:], ops[0:4, ds(0, BPC)])
            nc.sync.dma_start(outdev[:, :], out_sb[:])

        ctx.close()

    nc.compile()
    return nc


def _prep_inputs(x, ei, host_idx, W1, b1, W2, b2, Wout, bout):
    """Host-side: dense per-graph adjacency, packed layouts, sharding.
    Returns (in_maps, bout_val) or None if structural assumptions fail."""
    x = np.asarray(x); ei = np.asarray(ei); host_idx = np.asarray(host_idx)
    W1 = np.asarray(W1); b1 = np.asarray(b1); W2 = np.asarray(W2)
    b2 = np.asarray(b2); Wout = np.asarray(Wout); bout = np.asarray(bout)

    N = NG * NPG
    src = ei[0].astype(np.int64)
    dst = ei[1].astype(np.int64)
    if (src // NPG != dst // NPG).any():
        return None
    hi = host_idx.reshape(NG, NH)
    if not (hi == (np.arange(NG)[:, None] * NPG + np.arange(NH)[None, :])).all():
        return None
    if b1.any() or b2.any():
        return None

    deg = np.bincount(dst, minlength=N).astype(np.float64) + 1.0
    dinv = 1.0 / np.sqrt(deg)
    A = np.zeros((NG, NPG, NPG), dtype=np.float64)
    g = src // NPG
    np.add.at(A, (g, dst % NPG, src % NPG), dinv[src] * dinv[dst])
    A[:, np.arange(NPG), np.arange(NPG)] += (dinv * dinv).reshape(NG, NPG)
    A32 = A.astype(np.float32)

    WoutR = Wout[:, 0].reshape(NH, H2).astype(np.float32)
    mask = np.zeros((128, H2), dtype=np.float32)
    for p in range(128):
        if p % NPG < NH:
            mask[p] = WoutR[p % NPG]
    g4 = np.zeros((128, 4), dtype=np.float32)
    for p in range(128):
        g4[p, p // 32] = 1.0

    w2re = np.empty((128, 2 * H2), dtype=np.float32)
    w2re[:, :H2] = W2[:128]
    w2re[:, H2:] = W2[128:]

    in_maps = []
    for c in range(NCORES):
        mega = np.zeros((128, MEGA_W), dtype=np.float32)
        mega[:, OFF_W1:OFF_W1 + H1] = W1
        mega[:, OFF_W2:OFF_W2 + 2 * H2] = w2re
        mega[:, OFF_MASK:OFF_MASK + H2] = mask
        mega[:, OFF_G4:OFF_G4 + 4] = g4
        xc = x[c * NPC:(c + 1) * NPC].reshape(BPC, 128, IN_DIM)
        xc = np.ascontiguousarray(xc.transpose(1, 0, 2))       # [128, BPC, 128]
        Ac = A32[c * GPC:(c + 1) * GPC].reshape(BPC, 4, NPG, NPG)
        Ablk = np.zeros((BPC, 128, 128), dtype=np.float32)
        for j in range(4):
            # Ablk[b][s, d] = A[g][d_local, s_local]  (transposed within graph)
            Ablk[:, 32 * j:32 * (j + 1), 32 * j:32 * (j + 1)] = \
                Ac[:, j].transpose(0, 2, 1)
        Ablk = np.ascontiguousarray(Ablk.transpose(1, 0, 2))   # [128, BPC, 128]
        for q in range(NCHUNK):
            bs = slice(q * CBLK, (q + 1) * CBLK)
            base = CONST_W + q * CSTRIDE
            mega[:, base:base + CA] = Ablk[:, bs].reshape(128, CA)
            mega[:, base + CA:base + CSTRIDE] = xc[:, bs].reshape(128, CA)
        in_maps.append({"megadev": mega.astype(BF16)})
    return in_maps, float(bout[0])


def _numpy_fallback(x, ei, host_idx, W1, b1, W2, b2, Wout, bout):
    import jax
    jax.config.update("jax_platforms", "cpu")
    import jax.numpy as jnp

    def gcn_conv(xx, eei, W, b):
        Nn = xx.shape[0]
        loop = jnp.arange(Nn, dtype=eei.dtype)
        s = jnp.concatenate([eei[0], loop])
        d = jnp.concatenate([eei[1], loop])
        deg = jax.ops.segment_sum(jnp.ones(d.shape, dtype=xx.dtype), d, num_segments=Nn)
        dinv = jnp.where(deg > 0, jax.lax.rsqrt(deg), 0.0)
        norm = dinv[s] * dinv[d]
        h = xx @ W
        agg = jax.ops.segment_sum(h[s] * norm[:, None], d, num_segments=Nn)
        return agg + b

    h = jax.nn.relu(gcn_conv(jnp.asarray(x), jnp.asarray(ei), jnp.asarray(W1), jnp.asarray(b1)))
    h = jax.nn.relu(gcn_conv(h, jnp.asarray(ei), jnp.asarray(W2), jnp.asarray(b2)))
    host_z = h[jnp.asarray(host_idx)]
    nb = host_idx.shape[0] // NH
    z = host_z.reshape(nb, NH * h.shape[1])
    return np.asarray(z @ jnp.asarray(Wout) + jnp.asarray(bout))


def kernel(**inputs):
    prep = _prep_inputs(**inputs)
    if prep is None:
        return _numpy_fallback(**inputs)
    in_maps, bout_val = prep

    from concourse.bass_utils import run_bass_kernel_spmd

    if "nc" not in _CACHE:
        _CACHE["nc"] = _build_bass()
    nc = _CACHE["nc"]

    res = run_bass_kernel_spmd(nc, in_maps, core_ids=list(range(NCORES)))
    out = np.empty((NG, 1), dtype=np.float32)
    for c in range(NCORES):
        o = res.results[c]["outdev"]          # [4, BPC]
        out[c * GPC:(c + 1) * GPC, 0] = o.T.ravel()
    out += bout_val
    return out


# revision 12
# speedup vs baseline: 1.3118x; 1.0287x over previous
"""Trainium2 Bass kernel for a GCN critic network (2x GCNConv + host readout).

Strategy: graphs are 32 nodes with no cross-graph edges, so the GCN
scatter-add is a dense 32x32 normalized-adjacency matmul per graph. Four
graphs pack into one 128x128 block-diagonal stationary operand for the
TensorEngine. Data-parallel over 8 cores (512 graphs / 128 blocks per core).

v4: chunked wavefront, software-pipelined. The per-core input is one DRAM
tensor ("mega") with the small constants FIRST, then chunks of blocks
([A slab | A-host slab | x slab]); each chunk has its own DMA so compute
starts as soon as chunk 0 lands. Tick t of the pipeline runs P2 of chunk
t-1, P1 of chunk t, P3 of t-2 and P4+readout of t-3 so every engine has
independent work and cross-engine deps are at least a tick old. First and
last chunks are half-size to shorten pipeline fill/drain.

Per chunk (bf16 matmuls, fp32 PSUM):
  P1: a0t[f,d] per block      (lhsT=x_b [s,f], rhs=Ablk_b [s,d])
  P2: h1t = relu(W1^T @ a0t)  (feat-major, W1 halves stationary)
  P3: g_b = h1_b @ W2         (node-major, h1t halves stationary)
  P4: host rows only: lhsT = A-host [s, 52], two blocks packed per PSUM
      tile at partition bases 0/64 -> [116, 64] per block pair
  RO: mm = max(psum,0)*Wout-mask (relu fused into the PSUM read), reduce
      per 64-col group -> v[hostrow, pair]; final sel13 matmul sums the 13
      host rows per graph on the PE.
PSUM evictions are the bottleneck (GPSIMD has no PSUM port); they are
split across Scalar (h1 relu, first half of g) and Vector (a0t, second
half of g, STT, reduce).
"""

from contextlib import ExitStack

import numpy as np
import ml_dtypes

NG = 4096
NPG = 32
NH = 13
IN_DIM = 128
H1 = 256
H2 = 64
NCORES = 8
GPC = NG // NCORES          # graphs per core = 512
BPC = GPC // 4              # blocks per core = 128 (4 graphs / block)
NPC = GPC * NPG             # nodes per core = 16384

HC = 4 * NH                 # host cols per block = 52
# chunk sizes in blocks (sum = BPC); small edges for faster fill/drain
CHUNKS = [4, 4] + [8] * 14 + [4, 4]
assert sum(CHUNKS) == BPC
NCHUNK = len(CHUNKS)
CBASE = [0] * NCHUNK        # first block of each chunk
for _i in range(1, NCHUNK):
    CBASE[_i] = CBASE[_i - 1] + CHUNKS[_i - 1]

# mega layout (elements per partition, bf16): consts first, then chunks
OFF_W1 = 0                  # [128, 256]
OFF_W2 = OFF_W1 + H1        # two [128,64] halves side by side
OFF_MASK = OFF_W2 + 2 * H2  # [104, 64] Wout rows by host index
OFF_SEL = OFF_MASK + H2     # [104, 8] host-group selector
CONST_W = 512
CHUNK_OFF = [0] * NCHUNK    # column offset of each chunk
_o = CONST_W
for _i in range(NCHUNK):
    CHUNK_OFF[_i] = _o
    _o += CHUNKS[_i] * (128 + HC + 128)
MEGA_W = _o

BF16 = ml_dtypes.bfloat16

_CACHE = {}


def _chunk_of(b):
    for c in range(NCHUNK):
        if b < CBASE[c] + CHUNKS[c]:
            return c
    raise ValueError(b)


def _a_off(b):
    c = _chunk_of(b)
    return CHUNK_OFF[c] + (b - CBASE[c]) * 128


def _ah_off(b):
    c = _chunk_of(b)
    return CHUNK_OFF[c] + CHUNKS[c] * 128 + (b - CBASE[c]) * HC


def _x_off(b):
    c = _chunk_of(b)
    return CHUNK_OFF[c] + CHUNKS[c] * (128 + HC) + (b - CBASE[c]) * 128


def _build_bass():
    import concourse.bass as bass
    import concourse.mybir as mybir
    import concourse.tile as tile
    from concourse import bacc
    from concourse.bass import ds

    bf = mybir.dt.bfloat16
    f32 = mybir.dt.float32
    AF = mybir.ActivationFunctionType
    ALU = mybir.AluOpType

    nc = bacc.Bacc("TRN2", target_bir_lowering=False, debug=False)

    megadev = nc.declare_dram_parameter("megadev", [128, MEGA_W], bf, isOutput=False)
    outdev = nc.declare_dram_parameter("outdev", [8, BPC // 2], f32, isOutput=True)

    with tile.TileContext(nc) as tc:
        ctx = ExitStack()
        ppool = ctx.enter_context(tc.tile_pool(name="persist", bufs=1))
        mega = ppool.tile([128, MEGA_W], bf, name="mega", tag="mega")
        v_all = ppool.tile([116, BPC // 2], f32, name="v_all", tag="v_all")
        vb = ppool.tile([116, BPC // 2], bf, name="vb", tag="vb")
        mask4 = ppool.tile([116, 4 * H2], bf, name="mask4", tag="mask4")
        out_sb = ppool.tile([8, BPC // 2], f32, name="out_sb", tag="out_sb")

        w1at = mega[:, ds(OFF_W1, 128)]
        w1bt = mega[:, ds(OFF_W1 + 128, 128)]
        w2at = mega[:, ds(OFF_W2, H2)]
        w2bt = mega[:, ds(OFF_W2 + H2, H2)]
        maskc = mega[0:116, ds(OFF_MASK, H2)]
        sel13 = mega[0:116, ds(OFF_SEL, 8)]

        # consts first, then one DMA per chunk, issued in consumption order
        nc.sync.dma_start(mega[:, ds(0, CONST_W)], megadev[:, ds(0, CONST_W)])
        for c in range(NCHUNK):
            off, n = CHUNK_OFF[c], CHUNKS[c] * (128 + HC + 128)
            nc.sync.dma_start(mega[:, ds(off, n)], megadev[:, ds(off, n)])

        with tc.tile_pool(name="ps", bufs=1, space="PSUM") as pspool, \
             tc.tile_pool(name="ev", bufs=2) as evpool:

            # replicate Wout-mask 4x (DVE, waits only on the consts DMA)
            for j in range(4):
                nc.vector.tensor_scalar_add(mask4[:, ds(j * H2, H2)], maskc, 0.0)

            # zero PSUM rows 52..63 of the pR bank once (pR rotates over one
            # buffer) so the zero-masked STT rows never read stale NaN bits
            ps_r0 = pspool.tile([128, 256], f32, tag="pR", bufs=1)
            nc.scalar.memzero(ps_r0[:, :])

            a0t_t = {}
            h1_t = {}
            g8_t = {}
            for t in range(NCHUNK + 3):
                # ---- P2(t-1): h1t halves; relu evictions chase the pairs ----
                if 1 <= t <= NCHUNK:
                    c = t - 1
                    nb = CHUNKS[c]
                    a0t = a0t_t.pop(c)
                    ps_h = pspool.tile([128, 2048], f32, tag="pH", bufs=1)
                    h1 = evpool.tile([128, 2048], bf, tag="h1")
                    h1_t[c] = h1
                    for grp in range(nb // 4):
                        nc.tensor.matmul(ps_h[:, ds(grp * 1024, 512)], lhsT=w1at,
                                         rhs=a0t[:, ds(grp * 512, 512)],
                                         start=True, stop=True)
                        nc.tensor.matmul(ps_h[:, ds(grp * 1024 + 512, 512)], lhsT=w1bt,
                                         rhs=a0t[:, ds(grp * 512, 512)],
                                         start=True, stop=True)
                        nc.scalar.activation(h1[:, ds(grp * 1024, 1024)],
                                             ps_h[:, ds(grp * 1024, 1024)], AF.Relu)

                # ---- P1(t): a0t per block ----
                if t < NCHUNK:
                    nb = CHUNKS[t]
                    ps_a = pspool.tile([128, 1024], f32, tag="pA", bufs=1)
                    for j in range(nb):
                        b = CBASE[t] + j
                        nc.tensor.matmul(
                            ps_a[:, ds(128 * j, 128)],
                            lhsT=mega[:, ds(_x_off(b), 128)],
                            rhs=mega[:, ds(_a_off(b), 128)],
                            start=True, stop=True,
                        )

                # ---- P3(t-2): g per block ----
                if 2 <= t <= NCHUNK + 1:
                    c = t - 2
                    nb = CHUNKS[c]
                    h1 = h1_t.pop(c)
                    ps_g = pspool.tile([128, 512], f32, tag="pG", bufs=1)
                    for j in range(nb):
                        grp, o = j // 4, j % 4
                        la = h1[:, ds(grp * 1024 + o * 128, 128)]
                        lb = h1[:, ds(grp * 1024 + 512 + o * 128, 128)]
                        nc.tensor.matmul(ps_g[:, ds(j * H2, H2)], lhsT=la, rhs=w2at,
                                         start=True, stop=False)
                        nc.tensor.matmul(ps_g[:, ds(j * H2, H2)], lhsT=lb, rhs=w2bt,
                                         start=False, stop=True)
                    g8 = evpool.tile([128, 512], bf, tag="g8")
                    g8_t[c] = g8
                    half = nb * 32
                    nc.scalar.copy(g8[:, ds(0, half)], ps_g[:, ds(0, half)])
                    nc.vector.tensor_scalar_add(g8[:, ds(half, half)],
                                                ps_g[:, ds(half, half)], 0.0)

                # ---- P4(t-3) host rows + readout ----
                if 3 <= t:
                    c = t - 3
                    nb = CHUNKS[c]
                    g8 = g8_t.pop(c)
                    ps_r = pspool.tile([128, 256], f32, tag="pR", bufs=1)
                    for j in range(nb):
                        b = CBASE[c] + j
                        po = (j % 2) * 64
                        nc.tensor.matmul(ps_r[po:po + HC, ds((j // 2) * H2, H2)],
                                         lhsT=mega[:, ds(_ah_off(b), HC)],
                                         rhs=g8[:, ds(j * H2, H2)],
                                         start=True, stop=True)
                    npair = nb // 2
                    mm = evpool.tile([116, 256], bf, tag="mm")
                    nc.vector.scalar_tensor_tensor(
                        mm[:, ds(0, npair * H2)], ps_r[0:116, ds(0, npair * H2)],
                        0.0, mask4[:, ds(0, npair * H2)],
                        op0=ALU.max, op1=ALU.mult,
                    )
                    nc.vector.tensor_reduce(
                        v_all[:, ds(CBASE[c] // 2, npair)],
                        mm[:, ds(0, npair * H2)].rearrange("p (k c) -> p k c", c=H2),
                        axis=mybir.AxisListType.X,
                        op=ALU.add,
                    )

                # ---- a0t eviction for P1(t), last on DVE ----
                if t < NCHUNK:
                    nb = CHUNKS[t]
                    a0t = evpool.tile([128, 1024], bf, tag="a0t")
                    a0t_t[t] = a0t
                    nc.vector.tensor_scalar_add(a0t[:, ds(0, nb * 128)],
                                                ps_a[:, ds(0, nb * 128)], 0.0)

            # ---- final: sum the 13 host rows per graph on the PE ----
            nc.vector.tensor_scalar_add(vb[:], v_all[:], 0.0)
            ops = pspool.tile([128, 256], f32, tag="pR", bufs=1)
            nc.tensor.matmul(ops[0:8, ds(0, BPC // 2)], lhsT=sel13, rhs=vb[:],
                             start=True, stop=True)
            nc.scalar.copy(out_sb[:], ops[0:8, ds(0, BPC // 2)])
            nc.sync.dma_start(outdev[:, :], out_sb[:])

        ctx.close()

    nc.compile()
    return nc


def _prep_inputs(x, ei, host_idx, W1, b1, W2, b2, Wout, bout):
    """Host-side: dense per-graph adjacency, packed layouts, sharding.
    Returns (in_maps, bout_val) or None if structural assumptions fail."""
    x = np.asarray(x); ei = np.asarray(ei); host_idx = np.asarray(host_idx)
    W1 = np.asarray(W1); b1 = np.asarray(b1); W2 = np.asarray(W2)
    b2 = np.asarray(b2); Wout = np.asarray(Wout); bout = np.asarray(bout)

    N = NG * NPG
    src = ei[0].astype(np.int64)
    dst = ei[1].astype(np.int64)
    if (src // NPG != dst // NPG).any():
        return None
    hi = host_idx.reshape(NG, NH)
    if not (hi == (np.arange(NG)[:, None] * NPG + np.arange(NH)[None, :])).all():
        return None
    if b1.any() or b2.any():
        return None

    deg = np.bincount(dst, minlength=N).astype(np.float64) + 1.0
    dinv = 1.0 / np.sqrt(deg)
    A = np.zeros((NG, NPG, NPG), dtype=np.float64)
    g = src // NPG
    np.add.at(A, (g, dst % NPG, src % NPG), dinv[src] * dinv[dst])
    A[:, np.arange(NPG), np.arange(NPG)] += (dinv * dinv).reshape(NG, NPG)
    A32 = A.astype(np.float32)

    # host rows of a block pair pack at partition bases 0 (even block) and
    # 64 (odd block); rows 52..63 are dead and zero-masked
    WoutR = Wout[:, 0].reshape(NH, H2).astype(np.float32)
    mask2 = np.zeros((116, H2), dtype=np.float32)
    sel = np.zeros((116, 8), dtype=np.float32)
    for p in range(116):
        q = p if p < HC else p - 64
        if 0 <= q < HC:
            mask2[p] = WoutR[q % NH]
            sel[p, (0 if p < HC else 4) + q // NH] = 1.0

    w2re = np.empty((128, 2 * H2), dtype=np.float32)
    w2re[:, :H2] = W2[:128]
    w2re[:, H2:] = W2[128:]

    hostcols = (np.arange(4)[:, None] * NPG + np.arange(NH)[None, :]).ravel()

    in_maps = []
    for c in range(NCORES):
        mega = np.zeros((128, MEGA_W), dtype=np.float32)
        mega[:, OFF_W1:OFF_W1 + H1] = W1
        mega[:, OFF_W2:OFF_W2 + 2 * H2] = w2re
        mega[0:116, OFF_MASK:OFF_MASK + H2] = mask2
        mega[0:116, OFF_SEL:OFF_SEL + 8] = sel
        xc = x[c * NPC:(c + 1) * NPC].reshape(BPC, 128, IN_DIM)
        xc = np.ascontiguousarray(xc.transpose(1, 0, 2))       # [128, BPC, 128]
        Ac = A32[c * GPC:(c + 1) * GPC].reshape(BPC, 4, NPG, NPG)
        Ablk = np.zeros((BPC, 128, 128), dtype=np.float32)
        for j in range(4):
            # Ablk[b][s, d] = A[g][d_local, s_local]  (transposed within graph)
            Ablk[:, 32 * j:32 * (j + 1), 32 * j:32 * (j + 1)] = \
                Ac[:, j].transpose(0, 2, 1)
        Ablk = np.ascontiguousarray(Ablk.transpose(1, 0, 2))   # [128, BPC, 128]
        Ah = Ablk[:, :, hostcols]                              # [128, BPC, 52]
        for q in range(NCHUNK):
            nb = CHUNKS[q]
            bs = slice(CBASE[q], CBASE[q] + nb)
            base = CHUNK_OFF[q]
            mega[:, base:base + nb * 128] = Ablk[:, bs].reshape(128, nb * 128)
            mega[:, base + nb * 128:base + nb * (128 + HC)] = \
                Ah[:, bs].reshape(128, nb * HC)
            mega[:, base + nb * (128 + HC):base + nb * (256 + HC)] = \
                xc[:, bs].reshape(128, nb * 128)
        in_maps.append({"megadev": mega.astype(BF16)})
    return in_maps, float(bout[0])


def _numpy_fallback(x, ei, host_idx, W1, b1, W2, b2, Wout, bout):
    import jax
    jax.config.update("jax_platforms", "cpu")
    import jax.numpy as jnp

    def gcn_conv(xx, eei, W, b):
        Nn = xx.shape[0]
        loop = jnp.arange(Nn, dtype=eei.dtype)
        s = jnp.concatenate([eei[0], loop])
        d = jnp.concatenate([eei[1], loop])
        deg = jax.ops.segment_sum(jnp.ones(d.shape, dtype=xx.dtype), d, num_segments=Nn)
        dinv = jnp.where(deg > 0, jax.lax.rsqrt(deg), 0.0)
        norm = dinv[s] * dinv[d]
        h = xx @ W
        agg = jax.ops.segment_sum(h[s] * norm[:, None], d, num_segments=Nn)
        return agg + b

    h = jax.nn.relu(gcn_conv(jnp.asarray(x), jnp.asarray(ei), jnp.asarray(W1), jnp.asarray(b1)))
    h = jax.nn.relu(gcn_conv(h, jnp.asarray(ei), jnp.asarray(W2), jnp.asarray(b2)))
    host_z = h[jnp.asarray(host_idx)]
    nb = host_idx.shape[0] // NH
    z = host_z.reshape(nb, NH * h.shape[1])
    return np.asarray(z @ jnp.asarray(Wout) + jnp.asarray(bout))


def kernel(**inputs):
    prep = _prep_inputs(**inputs)
    if prep is None:
        return _numpy_fallback(**inputs)
    in_maps, bout_val = prep

    from concourse.bass_utils import run_bass_kernel_spmd

    if "nc" not in _CACHE:
        _CACHE["nc"] = _build_bass()
    nc = _CACHE["nc"]

    res = run_bass_kernel_spmd(nc, in_maps, core_ids=list(range(NCORES)))
    out = np.empty((NG, 1), dtype=np.float32)
    for c in range(NCORES):
        o = res.results[c]["outdev"]          # [8, BPC//2]; graph = pair*8 + j
        out[c * GPC:(c + 1) * GPC, 0] = o.T.ravel()
    out += bout_val
    return out
